# revision 61
# baseline (speedup 1.0000x reference)
"""Self-contained Trainium2 Bass kernel for BoSs (block-of-states) attention.

Strategy (8 NeuronCores):
  - data-parallel over batch (2) x tensor-parallel over heads (4):
    core c handles batch c//4, q-heads [4g:4g+4] and kv-head g where g=c%4.
  - host sorts tokens by state id with states relabeled by descending segment
    length (so both batches produce the same padded block structure), then
    pads each segment to a multiple of 128.  In padded coordinates the BoSs
    mask is exactly: blocks within one segment, causal, with a single shared
    lower-triangle mask on diagonal blocks (plus a per-segment tail mask on
    the segment's last block).  The sliding window (1024) never binds since
    segments are ~280 tokens.
  - projections and the output GEMM run on the n-hat-packed (unpadded) token
    axis so no FLOPs are spent on padding.
  - fp8 (e4m3) DoubleRow matmuls with hi+lo error compensation for the q/k/v
    projections and the Wo GEMM: x = xh+xl, W = Wh+Wl (host-split after
    scaling into e4m3's sweet spot); the three cross terms xh*Wh, xh*Wl,
    xl*Wh are computed with paired-k-tile DoubleRow instructions (2 k-tiles
    per instruction at 0.5 cycles/row) -> 1.33x over fp16 at ~1e-3 accuracy.
  - attention (scores, softmax denominator, AV) stays fp16: its contraction
    depth (128) is too short for the pairing to pay for the extra casts.
  - global scales (inputs *8, weights *512) keep every fp8 split well above
    the e4m3 subnormal floor; the exp() activation scale and a final host
    divide undo them exactly.
"""

import numpy as np
from contextlib import ExitStack

# problem constants (hardcoded per spec)
B, L, HID = 2, 2048, 2048
H, KVH, D = 16, 4, 128
THETA = 10000.0
NCORES = 8
TP = 4            # tensor-parallel group size (cores per batch)
QH = H // TP      # q heads per core = 4
NHC = HID // 128  # 16 hidden-dim chunks
NSEG = 8
SCALE = float(D) ** -0.5

# fp8 scaling: values ~N(0, 8..10) sit mid-range in e4m3 so the hi/lo split
# residuals stay far above the subnormal floor (2^-9).
SX = 8.0
SW = 512.0
SWO = 512.0
V0 = 128.0        # folded into the softmax-denominator ones vector: oT = o/V0
SCALE_EFF = SCALE / (SW * SX) ** 2
OUT_DESCALE = V0 / (SWO * SW * SX)

_CACHE = {}
LAST_EXEC_NS = None
LAST_RUN_WALL_S = None


def _structure(sid):
    """Shared padded block structure from both batches' state histograms."""
    counts = []
    perms = []
    for b in range(B):
        s = np.asarray(sid[b]).astype(np.int64)
        n = np.bincount(s, minlength=NSEG)
        order = np.argsort(-n, kind="stable")       # states by length desc
        rank = np.empty(NSEG, np.int64)
        rank[order] = np.arange(NSEG)
        perm = np.argsort(rank[s], kind="stable")   # tokens by (rank, pos)
        counts.append(np.sort(n)[::-1])
        perms.append(perm)
    nhat = np.maximum(counts[0], counts[1])
    T = np.maximum(1, np.ceil(nhat / 128).astype(np.int64))
    assert nhat.max() <= 512, f"segment too long: {nhat.max()}"
    assert T.max() <= 4
    return tuple(int(t) for t in T), tuple(int(v) for v in nhat), perms, counts


def _build_nc(T, nhat):
    import concourse.tile as tile
    from concourse import bacc, mybir

    f32 = mybir.dt.float32
    f16 = mybir.dt.float16
    f8 = mybir.dt.float8e4
    EXP = mybir.ActivationFunctionType.Exp
    DR = mybir.MatmulPerfMode.DoubleRow

    NBLK = sum(T)
    LPAD = 128 * NBLK
    NPACK = int(sum(nhat))
    NT = (NPACK + 127) // 128          # Wo token tiles
    pbase = np.cumsum([0] + list(T)).tolist()
    nbase = np.cumsum([0] + list(nhat)).tolist()
    NMASK = 1 + NSEG

    nc = bacc.Bacc(
        "TRN2", target_bir_lowering=False, debug=False, num_devices=NCORES
    )

    x8_d = nc.dram_tensor("x8", [128, NHC, 2, NPACK], f8, kind="ExternalInput").ap()
    wq8_d = nc.dram_tensor("wq8", [128, NHC, 2, QH * D], f8, kind="ExternalInput").ap()
    wk8_d = nc.dram_tensor("wk8", [128, NHC, 2, D], f8, kind="ExternalInput").ap()
    wv8_d = nc.dram_tensor("wv8", [128, NHC, 2, D], f8, kind="ExternalInput").ap()
    wo8_d = [
        nc.dram_tensor(n, [128, QH, HID], f8, kind="ExternalInput").ap()
        for n in ("wo8h", "wo8l")
    ]
    cosd = nc.dram_tensor("cosd", [128, NPACK], f16, kind="ExternalInput").ap()
    sind = nc.dram_tensor("sind", [128, NPACK], f16, kind="ExternalInput").ap()
    trid = nc.dram_tensor("trid", [128, NMASK, 128], f16, kind="ExternalInput").ap()
    idnd = nc.dram_tensor("idnd", [128, 128], f16, kind="ExternalInput").ap()
    swpd = nc.dram_tensor("swpd", [128, 128], f16, kind="ExternalInput").ap()
    out = nc.dram_tensor("out", [NT * 128, HID], f16, kind="ExternalOutput").ap()

    with tile.TileContext(nc) as tc, ExitStack() as top:
        persist = top.enter_context(tc.tile_pool(name="persist", bufs=1))
        kT = persist.tile([128, LPAD], f16, tag="kT", name="kT")
        qT = [
            persist.tile([128, LPAD], f16, tag=f"qT{h}", name=f"qT{h}")
            for h in range(QH)
        ]
        vT = persist.tile([128, LPAD], f16, tag="vT", name="vT")
        vA = persist.tile([128, NBLK, 128], f16, tag="vA", name="vA")
        cosT = persist.tile([128, NPACK], f16, tag="cosT", name="cosT")
        sinT = persist.tile([128, NPACK], f16, tag="sinT", name="sinT")
        oh8 = persist.tile([128, QH, NT * 128], f8, tag="oh8", name="oh8")
        ol8 = persist.tile([128, QH, NT * 128], f8, tag="ol8", name="ol8")
        msk = persist.tile([128, NMASK, 128], f16, tag="msk", name="msk")
        ones = persist.tile([128, 1], f16, tag="ones", name="ones")
        ones1 = persist.tile([1, 128], f16, tag="ones1", name="ones1")
        swp = persist.tile([128, 128], f16, tag="swp", name="swp")
        idn = persist.tile([128, 128], f16, tag="idn", name="idn")

        wpool = top.enter_context(tc.tile_pool(name="wpool", bufs=1))
        x8 = wpool.tile([128, NHC, 2, NPACK], f8, tag="x8", name="x8")
        wq8 = wpool.tile([128, NHC, 2, QH * D], f8, tag="wq8", name="wq8")
        wk8 = wpool.tile([128, NHC, 2, D], f8, tag="wk8", name="wk8")
        wv8 = wpool.tile([128, NHC, 2, D], f8, tag="wv8", name="wv8")
        wo8 = [
            wpool.tile([128, QH, HID], f8, tag=f"wo8{i}", name=f"wo8{i}")
            for i in range(2)
        ]

        # ---- DMAs: ordered by first-use; x loaded per segment so the DMA
        # engines (a serial resource in the cost model) serve urgent weights
        # first.
        def xseg(s):
            c0, c1 = nbase[s], nbase[s + 1]
            nc.sync.dma_start(x8[:, :, :, c0:c1], x8_d[:, :, :, c0:c1])

        c1_0 = nbase[1]
        nc.sync.dma_start(wk8[:, :, 0, :], wk8_d[:, :, 0, :])
        nc.sync.dma_start(x8[:, :, 0, 0:c1_0], x8_d[:, :, 0, 0:c1_0])
        nc.sync.dma_start(wk8[:, :, 1, :], wk8_d[:, :, 1, :])
        nc.sync.dma_start(x8[:, :, 1, 0:c1_0], x8_d[:, :, 1, 0:c1_0])
        nc.sync.dma_start(wv8[:], wv8_d[:])
        nc.sync.dma_start(wq8[:, :, 0, :], wq8_d[:, :, 0, :])
        nc.sync.dma_start(wq8[:, :, 1, :], wq8_d[:, :, 1, :])
        nc.sync.dma_start(idn[:], idnd[:])
        xseg(1)
        nc.sync.dma_start(swp[:], swpd[:])
        nc.sync.dma_start(cosT[:], cosd[:])
        nc.sync.dma_start(sinT[:], sind[:])
        xseg(2)
        nc.sync.dma_start(msk[:], trid[:])
        for s in range(3, NSEG):
            xseg(s)
        nc.sync.dma_start(wo8[0][:], wo8_d[0][:])
        nc.sync.dma_start(wo8[1][:], wo8_d[1][:])

        # ones=V0 folds o's fp8 range scaling into the softmax denominator.
        nc.gpsimd.memset(ones[:], V0)
        nc.gpsimd.memset(ones1[:], 1.0)

        # zero the padded tails of kT/qT/vT so stale SBUF never reaches a
        # matmul (NaN bit patterns would poison even masked entries).
        mse = [nc.vector, nc.gpsimd]
        mi = 0
        for s in range(NSEG):
            w = int(nhat[s])
            p0 = pbase[s] * 128 + w
            p1 = pbase[s + 1] * 128
            if p1 > p0:
                for t in (kT, vT, *qT):
                    mse[mi % 2].memset(t[:, p0:p1], 0.0)
                    mi += 1

        plpool = top.enter_context(tc.tile_pool(name="plpool", bufs=6))
        tpool = top.enter_context(tc.tile_pool(name="tpool", bufs=3))
        ppool = top.enter_context(tc.tile_pool(name="ppool", bufs=5))
        spool = top.enter_context(tc.tile_pool(name="spool", bufs=4))
        obpool = top.enter_context(tc.tile_pool(name="obpool", bufs=2))
        # PSUM (8 banks): psP 2x[128,512] (proj + Wo), psS 3x[128,384]
        # (scores in 128-col slots; also the rope-swap staging in phase 1),
        # psLO 3x[128,256] (denominator at [0:1,128:256] -> 1/l broadcast at
        # [:,128:256] after the reciprocal is read, AV output at [:,0:128];
        # also the v-transpose staging in phase 1).
        assert max(T) <= 3
        assert max(nhat) <= 384
        psP = top.enter_context(tc.tile_pool(name="psP", bufs=2, space="PSUM"))
        psS = top.enter_context(tc.tile_pool(name="psS", bufs=3, space="PSUM"))
        psLO = top.enter_context(tc.tile_pool(name="psLO", bufs=3, space="PSUM"))

        # ---- phase 1: projections + rope (packed coords -> padded coords) ----
        def proj_accum(ps, w8, hb0, hb1, c0, c1):
            """ps[:, :W] += W^T x over all 16 k-tiles, fp8 compensated.
            Term order (hi*hi, lo*hi, hi*lo) delays the need for the lo
            tensors so their DMAs can trail the hi ones."""
            n = 0
            total = 3 * NHC // 2
            for wi, xi in ((0, 0), (1, 0), (0, 1)):
                for cp in range(0, NHC, 2):
                    lhsT = w8[:, cp : cp + 2, wi, hb0:hb1]
                    rhs = x8[:, cp : cp + 2, xi, c0:c1]
                    nc.tensor.matmul(
                        ps,
                        lhsT,
                        rhs,
                        start=(n == 0),
                        stop=(n == total - 1),
                        perf_mode=DR,
                    )
                    n += 1

        # emit order: all projection accumulations for a segment, with each
        # rope swap matmul deferred until after the next hb's projection so
        # PE never waits on the ACT plain-copy.
        pend = []  # deferred swap work: (plain, cols_packed, dst, pcol0, W)
        swctr = [0]

        def flush_swap():
            if not pend:
                return
            plain, c0, c1, dst, p0 = pend.pop(0)
            w = c1 - c0
            u = swctr[0]
            swctr[0] += 1
            sw = psS.tile([128, 384], f32, tag="S", name=f"sw{u}")
            nc.tensor.matmul(
                sw[:, :w], swp[:], plain[:, :w], start=True, stop=True
            )
            t1 = tpool.tile([128, 512], f16, tag="t1", name=f"t1_{u}")
            nc.gpsimd.tensor_mul(t1[:, :w], plain[:, :w], cosT[:, c0:c1])
            t2 = tpool.tile([128, 512], f16, tag="t2", name=f"t2_{u}")
            nc.vector.tensor_mul(t2[:, :w], sw[:, :w], sinT[:, c0:c1])
            nc.gpsimd.tensor_add(dst[:, p0 : p0 + w], t1[:, :w], t2[:, :w])

        # pass 1: k and v projections for all segments (small weights + the
        # per-segment x pieces arrive at the same rate PE consumes them),
        # then the q heads (wq has the whole k/v pass to arrive).
        def proj_one(s, hb):
            W = int(nhat[s])
            c0, c1 = nbase[s], nbase[s] + W
            p0 = pbase[s] * 128
            ps = psP.tile([128, 512], f32, tag="ps", name=f"ps{s}_{hb}")
            if hb == "k":
                proj_accum(ps[:, :W], wk8, 0, D, c0, c1)
            elif hb == "v":
                proj_accum(ps[:, :W], wv8, 0, D, c0, c1)
            else:
                proj_accum(ps[:, :W], wq8, hb * D, (hb + 1) * D, c0, c1)
            if hb == "v":
                nc.scalar.copy(vT[:, p0 : p0 + W], ps[:, :W])
                for i in range(T[s]):
                    kb = pbase[s] + i
                    vt = psLO.tile([128, 256], f32, tag="lo", name=f"vt{kb}")
                    nc.tensor.matmul(
                        vt[:, :128],
                        vT[:, kb * 128 : (kb + 1) * 128],
                        idn[:],
                        start=True,
                        stop=True,
                    )
                    nc.scalar.copy(vA[:, kb, :], vt[:, :128])
            else:
                plain = plpool.tile(
                    [128, 512], f16, tag="plain", name=f"pl{s}_{hb}"
                )
                nc.scalar.copy(plain[:, :W], ps[:, :W])
                dst = kT if hb == "k" else qT[hb]
                pend.append((plain, c0, c1, dst, p0))
                if len(pend) > 1:
                    flush_swap()

        for s in range(NSEG):
            proj_one(s, "k")
            proj_one(s, "v")
            for hb in range(QH):
                proj_one(s, hb)
        flush_swap()
        flush_swap()

        # ---- phase 2: segment-blocked attention (padded coords) ----
        def cp(eng, out_ap, in_ap):
            if eng is nc.scalar:
                eng.copy(out_ap, in_ap)
            else:
                eng.tensor_copy(out_ap, in_ap)

        eng_oh = [nc.scalar, nc.vector]
        eng_ol = [nc.gpsimd, nc.vector]
        eng_ob = [nc.scalar, nc.vector]
        wo_next = [0]  # next Wo token-tile to emit

        def emit_wo(ready_cols):
            """Emit Wo tiles whose oh8/ol8 inputs are complete."""
            while wo_next[0] < NT:
                tb = wo_next[0]
                w = min(128, NPACK - tb * 128)
                if tb * 128 + w > ready_cols:
                    return
                t0 = tb * 128
                ob = obpool.tile([128, HID], f16, tag="ob", name=f"ob{tb}")
                for hc in range(HID // 512):
                    f_ps = psP.tile([128, 512], f32, tag="ps", name=f"f{tb}_{hc}")
                    n = 0
                    for oi, wi in ((0, 0), (0, 1), (1, 0)):
                        o8 = oh8 if oi == 0 else ol8
                        w8 = wo8[wi]
                        for hp in (0, 2):
                            nc.tensor.matmul(
                                f_ps[:w, :],
                                o8[:, hp : hp + 2, t0 : t0 + w],
                                w8[:, hp : hp + 2, hc * 512 : (hc + 1) * 512],
                                start=(n == 0),
                                stop=(n == 5),
                                perf_mode=DR,
                            )
                            n += 1
                    cp(eng_ob[hc % 2], ob[:w, hc * 512 : (hc + 1) * 512], f_ps[:w, :])
                    nc.sync.dma_start(
                        out[t0 : t0 + w, hc * 512 : (hc + 1) * 512],
                        ob[:w, hc * 512 : (hc + 1) * 512],
                    )
                wo_next[0] += 1

        # attention units: the first two blocks of each segment fuse into one
        # 256-wide unit (identical matmul cycles via sub-range accumulation,
        # but half the softmax/normalize/fp8-split ops); a T=3 segment adds a
        # narrow single-block unit for its tail.
        units = []
        for s in range(NSEG):
            rem = int(nhat[s]) - (T[s] - 1) * 128
            tidx = 0 if rem == 128 else 1 + s
            if T[s] == 1:
                units.append((s, "one", tidx))
            else:
                units.append((s, "pair", 0 if T[s] > 2 else tidx))
                if T[s] == 3:
                    units.append((s, "tail", tidx))

        work = [(u, h) for u in range(len(units)) for h in range(QH)]
        state = {}

        def stage_a(idx):
            """scores -> exp -> diagonal masks"""
            u, h = work[idx]
            s, kind, midx = units[u]
            pb = pbase[s]
            s_ps = psS.tile([128, 512], f32, tag="S", name=f"s{u}_{h}")
            P = ppool.tile([128, 512], f16, tag="P", name=f"p{u}_{h}")
            if kind == "pair":
                # slot A: k-block 0 x q cols 0:256 ; slot B: k-block 1 x
                # q cols 128:256 (its first q-half is entirely masked out)
                nc.tensor.matmul(
                    s_ps[:, 0:256],
                    kT[:, pb * 128 : (pb + 1) * 128],
                    qT[h][:, pb * 128 : pb * 128 + 256],
                    start=True,
                    stop=True,
                )
                nc.tensor.matmul(
                    s_ps[:, 384:512],
                    kT[:, (pb + 1) * 128 : (pb + 2) * 128],
                    qT[h][:, (pb + 1) * 128 : (pb + 2) * 128],
                    start=True,
                    stop=True,
                )
                nc.scalar.activation(
                    P[:, 0:256], s_ps[:, 0:256], EXP, scale=SCALE_EFF
                )
                nc.scalar.activation(
                    P[:, 384:512], s_ps[:, 384:512], EXP, scale=SCALE_EFF
                )
                nc.vector.tensor_mul(P[:, 0:128], P[:, 0:128], msk[:, 0, :])
                nc.vector.tensor_mul(
                    P[:, 384:512], P[:, 384:512], msk[:, midx, :]
                )
            else:
                nkb = T[s] if kind == "tail" else 1
                jj = pb + (2 if kind == "tail" else 0)
                for ib in range(nkb):
                    kb = pb + ib
                    nc.tensor.matmul(
                        s_ps[:, ib * 128 : (ib + 1) * 128],
                        kT[:, kb * 128 : (kb + 1) * 128],
                        qT[h][:, jj * 128 : (jj + 1) * 128],
                        start=True,
                        stop=True,
                    )
                nc.scalar.activation(
                    P[:, : nkb * 128], s_ps[:, : nkb * 128], EXP, scale=SCALE_EFF
                )
                nc.vector.tensor_mul(
                    P[:, (nkb - 1) * 128 : nkb * 128],
                    P[:, (nkb - 1) * 128 : nkb * 128],
                    msk[:, midx, :],
                )
            state[idx] = P

        def stage_b(idx):
            """denominator + AV accumulation + reciprocal"""
            u, h = work[idx]
            s, kind, midx = units[u]
            pb = pbase[s]
            P = state[idx]
            lo = psLO.tile([128, 512], f32, tag="lo", name=f"lo{u}_{h}")
            if kind == "pair":
                nc.tensor.matmul(
                    lo[0:1, 256:512], ones[:], P[:, 0:256], start=True, stop=False
                )
                nc.tensor.matmul(
                    lo[0:1, 384:512],
                    ones[:],
                    P[:, 384:512],
                    start=False,
                    stop=True,
                    skip_group_check=True,
                )
                nc.tensor.matmul(
                    lo[:, 0:256],
                    vA[:, pb, :],
                    P[:, 0:256],
                    start=True,
                    stop=False,
                )
                nc.tensor.matmul(
                    lo[:, 128:256],
                    vA[:, pb + 1, :],
                    P[:, 384:512],
                    start=False,
                    stop=True,
                    skip_group_check=True,
                )
                rw = 256
            else:
                nkb = T[s] if kind == "tail" else 1
                for ib in range(nkb):
                    nc.tensor.matmul(
                        lo[0:1, 256:384],
                        ones[:],
                        P[:, ib * 128 : (ib + 1) * 128],
                        start=(ib == 0),
                        stop=(ib == nkb - 1),
                    )
                for ib in range(nkb):
                    nc.tensor.matmul(
                        lo[:, 0:128],
                        vA[:, pb + ib, :],
                        P[:, ib * 128 : (ib + 1) * 128],
                        start=(ib == 0),
                        stop=(ib == nkb - 1),
                    )
                rw = 128
            rc = spool.tile([1, 256], f32, tag="rc", name=f"rc{u}_{h}")
            nc.vector.reciprocal(rc[:, :rw], lo[0:1, 256 : 256 + rw])
            state[idx] = (lo, rc, rw)

        def stage_c(idx):
            """1/(V0*l) broadcast -> normalize -> fp8 hi/lo split"""
            u, h = work[idx]
            s, kind, midx = units[u]
            if kind == "pair":
                w = min(256, int(nhat[s]))
                nj0 = nbase[s]
            else:
                off = 256 if kind == "tail" else 0
                w = min(128, int(nhat[s]) - off)
                nj0 = nbase[s] + off
            lo, rc, rw = state.pop(idx)
            rb = spool.tile([128, 256], f32, tag="rb", name=f"rb{u}_{h}")
            nc.gpsimd.partition_broadcast(rb[:, :rw], rc[:, :rw])
            t16 = spool.tile([128, 256], f16, tag="t16", name=f"t16{u}_{h}")
            nc.vector.tensor_mul(t16[:, :w], lo[:, 0:w], rb[:, :w])
            cp(eng_oh[(u + h) % 2], oh8[:, h, nj0 : nj0 + w], t16[:, :w])
            eng_ol[(u + h) % 2].tensor_sub(
                ol8[:, h, nj0 : nj0 + w], t16[:, :w], oh8[:, h, nj0 : nj0 + w]
            )
            if h == QH - 1 and u >= 1:
                # Wo tiles fully covered by the PREVIOUS unit's columns (so
                # the fp8 o-splits they read are long since written)
                sP, kP, _ = units[u - 1]
                if kP == "pair" and T[sP] == 3:
                    nc_prev = nbase[sP] + 256
                else:
                    nc_prev = nbase[sP] + int(nhat[sP])
                emit_wo(nc_prev)

        LB, LC = 2, 4
        n_work = len(work)
        for idx in range(n_work + LC):
            if idx < n_work:
                stage_a(idx)
                flush_swap()
            if LB <= idx and idx - LB < n_work:
                stage_b(idx - LB)
            if LC <= idx and idx - LC < n_work:
                stage_c(idx - LC)
        emit_wo(NPACK)

    nc.compile()
    return nc


def _get_nc(T, nhat):
    key = (T, nhat)
    if key not in _CACHE:
        _CACHE[key] = _build_nc(T, nhat)
    return _CACHE[key]


def _split8(a):
    import ml_dtypes

    e4 = ml_dtypes.float8_e4m3
    hi = a.astype(e4)
    lo = (a - hi.astype(np.float32)).astype(e4)
    return hi, lo


def kernel(hidden_states, Wq, Wk, Wv, Wo, sid, position_ids):
    global LAST_EXEC_NS, LAST_RUN_WALL_S
    import time

    from concourse.bass_utils import run_bass_kernel_spmd

    hidden = np.asarray(hidden_states, dtype=np.float32)
    Wq = np.asarray(Wq, dtype=np.float32)
    Wk = np.asarray(Wk, dtype=np.float32)
    Wv = np.asarray(Wv, dtype=np.float32)
    Wo = np.asarray(Wo, dtype=np.float32)
    sid = np.asarray(sid)
    position_ids = np.asarray(position_ids)

    T, nhat, perms, counts = _structure(sid)
    nc = _get_nc(T, nhat)

    NBLK = sum(T)
    NPACK = int(sum(nhat))
    NT = (NPACK + 127) // 128
    nbase = np.cumsum([0] + list(nhat)).tolist()
    NMASK = 1 + NSEG

    f16 = np.float16

    # constants shared by all cores
    swpn = np.zeros((128, 128), f16)
    swpn[(np.arange(128) + 64) % 128, np.arange(128)] = 1.0
    idnn = np.eye(128, dtype=f16)
    ki = np.arange(128)[:, None]
    qi = np.arange(128)[None, :]
    tri = (ki <= qi).astype(f16)
    trin = np.zeros((128, NMASK, 128), f16)
    trin[:, 0, :] = tri
    for s in range(NSEG):
        rem = int(nhat[s]) - (T[s] - 1) * 128
        trin[:, 1 + s, :] = tri * (ki < rem)

    # weights per TP group (shared across batches)
    wgrp = []
    for g in range(TP):
        wq_dev = np.ascontiguousarray(
            (SW * Wq[g * 512 : (g + 1) * 512]).T
        ).reshape(NHC, 128, QH * D)
        wk_dev = np.ascontiguousarray(
            (SW * Wk[g * 128 : (g + 1) * 128]).T
        ).reshape(NHC, 128, D)
        wv_dev = np.ascontiguousarray(
            (SW * Wv[g * 128 : (g + 1) * 128]).T
        ).reshape(NHC, 128, D)
        # wo8[p, h, n] = SWO * Wo[n, g*512 + h*128 + p]
        wo_dev = np.ascontiguousarray(
            (SWO * Wo[:, g * 512 : (g + 1) * 512]).T.reshape(QH, 128, HID)
        ).transpose(1, 0, 2)
        ws = {}
        for name, a in (("wq8", wq_dev), ("wk8", wk_dev), ("wv8", wv_dev)):
            hi, lo = _split8(np.ascontiguousarray(a.transpose(1, 0, 2)))
            ws[name] = np.ascontiguousarray(np.stack([hi, lo], axis=2))
        hi, lo = _split8(np.ascontiguousarray(wo_dev))
        ws["wo8h"], ws["wo8l"] = hi, lo
        wgrp.append(ws)

    in_maps = []
    real_rows = []
    for b in range(B):
        perm = perms[b]
        n_b = counts[b]
        # n-hat-packed x with zero fill between n_b and nhat
        xs = hidden[b].T[:, perm]  # [HID, L] sorted
        xpack = np.zeros((HID, NPACK), np.float32)
        pos = np.zeros(NPACK, np.float32)
        rows = []
        off = 0
        for s in range(NSEG):
            w = int(n_b[s])
            xpack[:, nbase[s] : nbase[s] + w] = xs[:, off : off + w] * SX
            pos[nbase[s] : nbase[s] + w] = position_ids[b][
                perm[off : off + w]
            ].astype(np.float32)
            rows.append(nbase[s] + np.arange(w))
            off += w
        real_rows.append(np.concatenate(rows))

        x8h, x8l = _split8(
            np.ascontiguousarray(xpack.reshape(NHC, 128, NPACK).transpose(1, 0, 2))
        )
        x8p = np.ascontiguousarray(np.stack([x8h, x8l], axis=2))

        inv = 1.0 / (
            THETA ** (np.arange(0, D, 2, dtype=np.float32) / np.float32(D))
        )
        fr = pos[:, None] * inv[None, :]
        emb = np.concatenate([fr, fr], axis=1)  # [NPACK, D]
        cosT = np.ascontiguousarray(np.cos(emb).T.astype(f16))
        sinT = np.sin(emb).T.astype(np.float32).copy()
        sinT[: D // 2] *= -1.0  # fold rotate_half sign
        sinT = np.ascontiguousarray(sinT.astype(f16))

        for g in range(TP):
            m = dict(
                x8=x8p,
                cosd=cosT,
                sind=sinT,
                trid=trin,
                idnd=idnn,
                swpd=swpn,
            )
            m.update(wgrp[g])
            in_maps.append(m)

    t0 = time.time()
    res = run_bass_kernel_spmd(nc, in_maps, core_ids=list(range(NCORES)))
    LAST_RUN_WALL_S = time.time() - t0
    LAST_EXEC_NS = res.exec_time_ns

    full = np.empty((B, L, HID), np.float32)
    for b in range(B):
        acc = np.asarray(res.results[4 * b]["out"]).astype(np.float32)
        for g in range(1, TP):
            acc += np.asarray(res.results[4 * b + g]["out"]).astype(np.float32)
        unp = np.empty((L, HID), np.float32)
        unp[perms[b]] = acc[real_rows[b]]
        full[b] = unp * OUT_DESCALE
    return full


# revision 62
# speedup vs baseline: 1.0179x; 1.0179x over previous
"""Self-contained Trainium2 Bass kernel for BoSs (block-of-states) attention.

Strategy (8 NeuronCores):
  - data-parallel over batch (2) x tensor-parallel over heads (4):
    core c handles batch c//4, q-heads [4g:4g+4] and kv-head g where g=c%4.
  - host sorts tokens by state id with states relabeled by descending segment
    length (so both batches produce the same padded block structure), then
    pads each segment to a multiple of 128.  In padded coordinates the BoSs
    mask is exactly: blocks within one segment, causal, with a single shared
    lower-triangle mask on diagonal blocks (plus a per-segment tail mask on
    the segment's last block).  The sliding window (1024) never binds since
    segments are ~280 tokens.
  - projections and the output GEMM run on the n-hat-packed (unpadded) token
    axis so no FLOPs are spent on padding.
  - fp8 (e4m3) DoubleRow matmuls with hi+lo error compensation for the q/k/v
    projections and the Wo GEMM: x = xh+xl, W = Wh+Wl (host-split after
    scaling into e4m3's sweet spot); the three cross terms xh*Wh, xh*Wl,
    xl*Wh are computed with paired-k-tile DoubleRow instructions (2 k-tiles
    per instruction at 0.5 cycles/row) -> 1.33x over fp16 at ~1e-3 accuracy.
  - attention (scores, softmax denominator, AV) stays fp16: its contraction
    depth (128) is too short for the pairing to pay for the extra casts.
  - global scales (inputs *8, weights *512) keep every fp8 split well above
    the e4m3 subnormal floor; the exp() activation scale and a final host
    divide undo them exactly.
"""

import numpy as np
from contextlib import ExitStack

# problem constants (hardcoded per spec)
B, L, HID = 2, 2048, 2048
H, KVH, D = 16, 4, 128
THETA = 10000.0
NCORES = 8
TP = 4            # tensor-parallel group size (cores per batch)
QH = H // TP      # q heads per core = 4
NHC = HID // 128  # 16 hidden-dim chunks
NSEG = 8
SCALE = float(D) ** -0.5

# fp8 scaling: values ~N(0, 8..10) sit mid-range in e4m3 so the hi/lo split
# residuals stay far above the subnormal floor (2^-9).
SX = 8.0
SW = 512.0
SWO = 512.0
V0 = 128.0        # folded into the softmax-denominator ones vector: oT = o/V0
SCALE_EFF = SCALE / (SW * SX) ** 2
OUT_DESCALE = V0 / (SWO * SW * SX)

_CACHE = {}
LAST_EXEC_NS = None
LAST_RUN_WALL_S = None


def _structure(sid):
    """Shared padded block structure from both batches' state histograms."""
    counts = []
    perms = []
    for b in range(B):
        s = np.asarray(sid[b]).astype(np.int64)
        n = np.bincount(s, minlength=NSEG)
        order = np.argsort(-n, kind="stable")       # states by length desc
        rank = np.empty(NSEG, np.int64)
        rank[order] = np.arange(NSEG)
        perm = np.argsort(rank[s], kind="stable")   # tokens by (rank, pos)
        counts.append(np.sort(n)[::-1])
        perms.append(perm)
    nhat = np.maximum(counts[0], counts[1])
    T = np.maximum(1, np.ceil(nhat / 128).astype(np.int64))
    assert nhat.max() <= 512, f"segment too long: {nhat.max()}"
    assert T.max() <= 4
    return tuple(int(t) for t in T), tuple(int(v) for v in nhat), perms, counts


def _build_nc(T, nhat):
    import concourse.tile as tile
    from concourse import bacc, mybir

    f32 = mybir.dt.float32
    f16 = mybir.dt.float16
    f8 = mybir.dt.float8e4
    EXP = mybir.ActivationFunctionType.Exp
    DR = mybir.MatmulPerfMode.DoubleRow

    NBLK = sum(T)
    LPAD = 128 * NBLK
    NPACK = int(sum(nhat))
    NT = (NPACK + 127) // 128          # Wo token tiles
    pbase = np.cumsum([0] + list(T)).tolist()
    nbase = np.cumsum([0] + list(nhat)).tolist()
    NMASK = 1 + NSEG

    nc = bacc.Bacc(
        "TRN2", target_bir_lowering=False, debug=False, num_devices=NCORES
    )

    x8h_d = nc.dram_tensor("x8h", [128, NHC, NPACK], f8, kind="ExternalInput").ap()
    x8l_d = nc.dram_tensor("x8l", [128, NHC, NPACK], f8, kind="ExternalInput").ap()
    wq8_d = [
        nc.dram_tensor(n, [128, NHC, QH * D], f8, kind="ExternalInput").ap()
        for n in ("wq8h", "wq8l")
    ]
    wk8_d = [
        nc.dram_tensor(n, [128, NHC, D], f8, kind="ExternalInput").ap()
        for n in ("wk8h", "wk8l")
    ]
    wv8_d = [
        nc.dram_tensor(n, [128, NHC, D], f8, kind="ExternalInput").ap()
        for n in ("wv8h", "wv8l")
    ]
    wo8_d = [
        nc.dram_tensor(n, [128, QH, HID], f8, kind="ExternalInput").ap()
        for n in ("wo8h", "wo8l")
    ]
    cosd = nc.dram_tensor("cosd", [128, NPACK], f16, kind="ExternalInput").ap()
    sind = nc.dram_tensor("sind", [128, NPACK], f16, kind="ExternalInput").ap()
    trid = nc.dram_tensor("trid", [128, NMASK, 128], f16, kind="ExternalInput").ap()
    idnd = nc.dram_tensor("idnd", [128, 128], f16, kind="ExternalInput").ap()
    swpd = nc.dram_tensor("swpd", [128, 128], f16, kind="ExternalInput").ap()
    out = nc.dram_tensor("out", [NT * 128, HID], f16, kind="ExternalOutput").ap()

    with tile.TileContext(nc) as tc, ExitStack() as top:
        persist = top.enter_context(tc.tile_pool(name="persist", bufs=1))
        kT = persist.tile([128, LPAD], f16, tag="kT", name="kT")
        qT = [
            persist.tile([128, LPAD], f16, tag=f"qT{h}", name=f"qT{h}")
            for h in range(QH)
        ]
        vT = persist.tile([128, LPAD], f16, tag="vT", name="vT")
        vA = persist.tile([128, NBLK, 128], f16, tag="vA", name="vA")
        cosT = persist.tile([128, NPACK], f16, tag="cosT", name="cosT")
        sinT = persist.tile([128, NPACK], f16, tag="sinT", name="sinT")
        oh8 = persist.tile([128, QH, NT * 128], f8, tag="oh8", name="oh8")
        ol8 = persist.tile([128, QH, NT * 128], f8, tag="ol8", name="ol8")
        msk = persist.tile([128, NMASK, 128], f16, tag="msk", name="msk")
        ones = persist.tile([128, 1], f16, tag="ones", name="ones")
        ones1 = persist.tile([1, 128], f16, tag="ones1", name="ones1")
        swp = persist.tile([128, 128], f16, tag="swp", name="swp")
        idn = persist.tile([128, 128], f16, tag="idn", name="idn")

        wpool = top.enter_context(tc.tile_pool(name="wpool", bufs=1))
        x8h = wpool.tile([128, NHC, NPACK], f8, tag="x8h", name="x8h")
        x8l = wpool.tile([128, NHC, NPACK], f8, tag="x8l", name="x8l")
        wq8 = [
            wpool.tile([128, NHC, QH * D], f8, tag=f"wq8{i}", name=f"wq8{i}")
            for i in range(2)
        ]
        wk8 = [
            wpool.tile([128, NHC, D], f8, tag=f"wk8{i}", name=f"wk8{i}")
            for i in range(2)
        ]
        wv8 = [
            wpool.tile([128, NHC, D], f8, tag=f"wv8{i}", name=f"wv8{i}")
            for i in range(2)
        ]
        wo8 = [
            wpool.tile([128, QH, HID], f8, tag=f"wo8{i}", name=f"wo8{i}")
            for i in range(2)
        ]

        # ---- DMAs: ordered by first-use; x loaded per segment so the DMA
        # engines (a serial resource in the cost model) serve urgent weights
        # first.
        def xseg(s, split=False):
            c0, c1 = nbase[s], nbase[s + 1]
            groups = ((0, 8), (8, 16)) if split else ((0, 16),)
            for t in (x8h, x8l):
                td = x8h_d if t is x8h else x8l_d
                for g0, g1 in groups:
                    nc.sync.dma_start(t[:, g0:g1, c0:c1], td[:, g0:g1, c0:c1])

        nc.sync.dma_start(wk8[0][:], wk8_d[0][:])
        nc.sync.dma_start(x8h[:, :, nbase[0] : nbase[1]], x8h_d[:, :, nbase[0] : nbase[1]])
        nc.sync.dma_start(wk8[1][:], wk8_d[1][:])
        nc.sync.dma_start(x8l[:, :, nbase[0] : nbase[1]], x8l_d[:, :, nbase[0] : nbase[1]])
        nc.sync.dma_start(wv8[0][:], wv8_d[0][:])
        nc.sync.dma_start(wv8[1][:], wv8_d[1][:])
        nc.sync.dma_start(wq8[0][:], wq8_d[0][:])
        nc.sync.dma_start(wq8[1][:], wq8_d[1][:])
        nc.sync.dma_start(idn[:], idnd[:])
        xseg(1)
        nc.sync.dma_start(swp[:], swpd[:])
        nc.sync.dma_start(cosT[:], cosd[:])
        nc.sync.dma_start(sinT[:], sind[:])
        xseg(2)
        nc.sync.dma_start(msk[:], trid[:])
        for s in range(3, NSEG):
            xseg(s)
        nc.sync.dma_start(wo8[0][:], wo8_d[0][:])
        nc.sync.dma_start(wo8[1][:], wo8_d[1][:])

        # ones=V0 folds o's fp8 range scaling into the softmax denominator.
        nc.gpsimd.memset(ones[:], V0)
        nc.gpsimd.memset(ones1[:], 1.0)

        # zero the padded tails of kT/qT/vT so stale SBUF never reaches a
        # matmul (NaN bit patterns would poison even masked entries).
        mse = [nc.vector, nc.gpsimd]
        mi = 0
        for s in range(NSEG):
            w = int(nhat[s])
            p0 = pbase[s] * 128 + w
            p1 = pbase[s + 1] * 128
            if p1 > p0:
                for t in (kT, vT, *qT):
                    mse[mi % 2].memset(t[:, p0:p1], 0.0)
                    mi += 1

        plpool = top.enter_context(tc.tile_pool(name="plpool", bufs=6))
        tpool = top.enter_context(tc.tile_pool(name="tpool", bufs=3))
        ppool = top.enter_context(tc.tile_pool(name="ppool", bufs=5))
        spool = top.enter_context(tc.tile_pool(name="spool", bufs=4))
        obpool = top.enter_context(tc.tile_pool(name="obpool", bufs=2))
        # PSUM (8 banks): psP 2x[128,512] (proj + Wo), psS 3x[128,384]
        # (scores in 128-col slots; also the rope-swap staging in phase 1),
        # psLO 3x[128,256] (denominator at [0:1,128:256] -> 1/l broadcast at
        # [:,128:256] after the reciprocal is read, AV output at [:,0:128];
        # also the v-transpose staging in phase 1).
        assert max(T) <= 3
        assert max(nhat) <= 384
        psP = top.enter_context(tc.tile_pool(name="psP", bufs=2, space="PSUM"))
        psS = top.enter_context(tc.tile_pool(name="psS", bufs=3, space="PSUM"))
        psLO = top.enter_context(tc.tile_pool(name="psLO", bufs=3, space="PSUM"))

        # ---- phase 1: projections + rope (packed coords -> padded coords) ----
        def proj_accum(ps, w8, hb0, hb1, c0, c1):
            """ps[:, :W] += W^T x over all 16 k-tiles, fp8 compensated.
            Term order (hi*hi, lo*hi, hi*lo) delays the need for the lo
            tensors so their DMAs can trail the hi ones."""
            n = 0
            total = 3 * NHC // 2
            for wi, xi in ((0, 0), (1, 0), (0, 1)):
                for cp in range(0, NHC, 2):
                    lhsT = w8[wi][:, cp : cp + 2, hb0:hb1]
                    rhs = (x8h if xi == 0 else x8l)[:, cp : cp + 2, c0:c1]
                    nc.tensor.matmul(
                        ps,
                        lhsT,
                        rhs,
                        start=(n == 0),
                        stop=(n == total - 1),
                        perf_mode=DR,
                    )
                    n += 1

        # emit order: all projection accumulations for a segment, with each
        # rope swap matmul deferred until after the next hb's projection so
        # PE never waits on the ACT plain-copy.
        pend = []  # deferred swap work: (plain, cols_packed, dst, pcol0, W)
        swctr = [0]

        def flush_swap():
            if not pend:
                return
            plain, c0, c1, dst, p0 = pend.pop(0)
            w = c1 - c0
            u = swctr[0]
            swctr[0] += 1
            sw = psS.tile([128, 384], f32, tag="S", name=f"sw{u}")
            nc.tensor.matmul(
                sw[:, :w], swp[:], plain[:, :w], start=True, stop=True
            )
            t1 = tpool.tile([128, 512], f16, tag="t1", name=f"t1_{u}")
            nc.gpsimd.tensor_mul(t1[:, :w], plain[:, :w], cosT[:, c0:c1])
            t2 = tpool.tile([128, 512], f16, tag="t2", name=f"t2_{u}")
            nc.vector.tensor_mul(t2[:, :w], sw[:, :w], sinT[:, c0:c1])
            nc.gpsimd.tensor_add(dst[:, p0 : p0 + w], t1[:, :w], t2[:, :w])

        # pass 1: k and v projections for all segments (small weights + the
        # per-segment x pieces arrive at the same rate PE consumes them),
        # then the q heads (wq has the whole k/v pass to arrive).
        def proj_one(s, hb):
            W = int(nhat[s])
            c0, c1 = nbase[s], nbase[s] + W
            p0 = pbase[s] * 128
            ps = psP.tile([128, 512], f32, tag="ps", name=f"ps{s}_{hb}")
            if hb == "k":
                proj_accum(ps[:, :W], wk8, 0, D, c0, c1)
            elif hb == "v":
                proj_accum(ps[:, :W], wv8, 0, D, c0, c1)
            else:
                proj_accum(ps[:, :W], wq8, hb * D, (hb + 1) * D, c0, c1)
            if hb == "v":
                nc.scalar.copy(vT[:, p0 : p0 + W], ps[:, :W])
                for i in range(T[s]):
                    kb = pbase[s] + i
                    vt = psLO.tile([128, 256], f32, tag="lo", name=f"vt{kb}")
                    nc.tensor.matmul(
                        vt[:, :128],
                        vT[:, kb * 128 : (kb + 1) * 128],
                        idn[:],
                        start=True,
                        stop=True,
                    )
                    nc.scalar.copy(vA[:, kb, :], vt[:, :128])
            else:
                plain = plpool.tile(
                    [128, 512], f16, tag="plain", name=f"pl{s}_{hb}"
                )
                nc.scalar.copy(plain[:, :W], ps[:, :W])
                dst = kT if hb == "k" else qT[hb]
                pend.append((plain, c0, c1, dst, p0))
                if len(pend) > 1:
                    flush_swap()

        for s in range(NSEG):
            proj_one(s, "k")
            proj_one(s, "v")
            for hb in range(QH):
                proj_one(s, hb)
        flush_swap()
        flush_swap()

        # ---- phase 2: segment-blocked attention (padded coords) ----
        def cp(eng, out_ap, in_ap):
            if eng is nc.scalar:
                eng.copy(out_ap, in_ap)
            else:
                eng.tensor_copy(out_ap, in_ap)

        eng_oh = [nc.scalar, nc.vector]
        eng_ol = [nc.gpsimd, nc.vector]
        eng_ob = [nc.scalar, nc.vector]
        wo_next = [0]  # next Wo token-tile to emit

        def emit_wo(ready_cols):
            """Emit Wo tiles whose oh8/ol8 inputs are complete."""
            while wo_next[0] < NT:
                tb = wo_next[0]
                w = min(128, NPACK - tb * 128)
                if tb * 128 + w > ready_cols:
                    return
                t0 = tb * 128
                ob = obpool.tile([128, HID], f16, tag="ob", name=f"ob{tb}")
                for hc in range(HID // 512):
                    f_ps = psP.tile([128, 512], f32, tag="ps", name=f"f{tb}_{hc}")
                    n = 0
                    for oi, wi in ((0, 0), (0, 1), (1, 0)):
                        o8 = oh8 if oi == 0 else ol8
                        w8 = wo8[wi]
                        for hp in (0, 2):
                            nc.tensor.matmul(
                                f_ps[:w, :],
                                o8[:, hp : hp + 2, t0 : t0 + w],
                                w8[:, hp : hp + 2, hc * 512 : (hc + 1) * 512],
                                start=(n == 0),
                                stop=(n == 5),
                                perf_mode=DR,
                            )
                            n += 1
                    cp(eng_ob[hc % 2], ob[:w, hc * 512 : (hc + 1) * 512], f_ps[:w, :])
                    nc.sync.dma_start(
                        out[t0 : t0 + w, hc * 512 : (hc + 1) * 512],
                        ob[:w, hc * 512 : (hc + 1) * 512],
                    )
                wo_next[0] += 1

        # attention units: the first two blocks of each segment fuse into one
        # 256-wide unit (identical matmul cycles via sub-range accumulation,
        # but half the softmax/normalize/fp8-split ops); a T=3 segment adds a
        # narrow single-block unit for its tail.
        units = []
        for s in range(NSEG):
            rem = int(nhat[s]) - (T[s] - 1) * 128
            tidx = 0 if rem == 128 else 1 + s
            if T[s] == 1:
                units.append((s, "one", tidx))
            else:
                units.append((s, "pair", 0 if T[s] > 2 else tidx))
                if T[s] == 3:
                    units.append((s, "tail", tidx))

        work = [(u, h) for u in range(len(units)) for h in range(QH)]
        state = {}

        def stage_a(idx):
            """scores -> exp -> diagonal masks"""
            u, h = work[idx]
            s, kind, midx = units[u]
            pb = pbase[s]
            s_ps = psS.tile([128, 512], f32, tag="S", name=f"s{u}_{h}")
            P = ppool.tile([128, 512], f16, tag="P", name=f"p{u}_{h}")
            if kind == "pair":
                # slot A: k-block 0 x q cols 0:256 ; slot B: k-block 1 x
                # q cols 128:256 (its first q-half is entirely masked out)
                nc.tensor.matmul(
                    s_ps[:, 0:256],
                    kT[:, pb * 128 : (pb + 1) * 128],
                    qT[h][:, pb * 128 : pb * 128 + 256],
                    start=True,
                    stop=True,
                )
                nc.tensor.matmul(
                    s_ps[:, 384:512],
                    kT[:, (pb + 1) * 128 : (pb + 2) * 128],
                    qT[h][:, (pb + 1) * 128 : (pb + 2) * 128],
                    start=True,
                    stop=True,
                )
                nc.scalar.activation(
                    P[:, 0:256], s_ps[:, 0:256], EXP, scale=SCALE_EFF
                )
                nc.scalar.activation(
                    P[:, 384:512], s_ps[:, 384:512], EXP, scale=SCALE_EFF
                )
                nc.vector.tensor_mul(P[:, 0:128], P[:, 0:128], msk[:, 0, :])
                nc.vector.tensor_mul(
                    P[:, 384:512], P[:, 384:512], msk[:, midx, :]
                )
            else:
                nkb = T[s] if kind == "tail" else 1
                jj = pb + (2 if kind == "tail" else 0)
                for ib in range(nkb):
                    kb = pb + ib
                    nc.tensor.matmul(
                        s_ps[:, ib * 128 : (ib + 1) * 128],
                        kT[:, kb * 128 : (kb + 1) * 128],
                        qT[h][:, jj * 128 : (jj + 1) * 128],
                        start=True,
                        stop=True,
                    )
                nc.scalar.activation(
                    P[:, : nkb * 128], s_ps[:, : nkb * 128], EXP, scale=SCALE_EFF
                )
                nc.vector.tensor_mul(
                    P[:, (nkb - 1) * 128 : nkb * 128],
                    P[:, (nkb - 1) * 128 : nkb * 128],
                    msk[:, midx, :],
                )
            state[idx] = P

        def stage_b(idx):
            """denominator + AV accumulation + reciprocal"""
            u, h = work[idx]
            s, kind, midx = units[u]
            pb = pbase[s]
            P = state[idx]
            lo = psLO.tile([128, 512], f32, tag="lo", name=f"lo{u}_{h}")
            if kind == "pair":
                nc.tensor.matmul(
                    lo[0:1, 256:512], ones[:], P[:, 0:256], start=True, stop=False
                )
                nc.tensor.matmul(
                    lo[0:1, 384:512],
                    ones[:],
                    P[:, 384:512],
                    start=False,
                    stop=True,
                    skip_group_check=True,
                )
                nc.tensor.matmul(
                    lo[:, 0:256],
                    vA[:, pb, :],
                    P[:, 0:256],
                    start=True,
                    stop=False,
                )
                nc.tensor.matmul(
                    lo[:, 128:256],
                    vA[:, pb + 1, :],
                    P[:, 384:512],
                    start=False,
                    stop=True,
                    skip_group_check=True,
                )
                rw = 256
            else:
                nkb = T[s] if kind == "tail" else 1
                for ib in range(nkb):
                    nc.tensor.matmul(
                        lo[0:1, 256:384],
                        ones[:],
                        P[:, ib * 128 : (ib + 1) * 128],
                        start=(ib == 0),
                        stop=(ib == nkb - 1),
                    )
                for ib in range(nkb):
                    nc.tensor.matmul(
                        lo[:, 0:128],
                        vA[:, pb + ib, :],
                        P[:, ib * 128 : (ib + 1) * 128],
                        start=(ib == 0),
                        stop=(ib == nkb - 1),
                    )
                rw = 128
            rc = spool.tile([1, 256], f32, tag="rc", name=f"rc{u}_{h}")
            nc.vector.reciprocal(rc[:, :rw], lo[0:1, 256 : 256 + rw])
            state[idx] = (lo, rc, rw)

        def stage_c(idx):
            """1/(V0*l) broadcast -> normalize -> fp8 hi/lo split"""
            u, h = work[idx]
            s, kind, midx = units[u]
            if kind == "pair":
                w = min(256, int(nhat[s]))
                nj0 = nbase[s]
            else:
                off = 256 if kind == "tail" else 0
                w = min(128, int(nhat[s]) - off)
                nj0 = nbase[s] + off
            lo, rc, rw = state.pop(idx)
            rb = spool.tile([128, 256], f32, tag="rb", name=f"rb{u}_{h}")
            nc.gpsimd.partition_broadcast(rb[:, :rw], rc[:, :rw])
            t16 = spool.tile([128, 256], f16, tag="t16", name=f"t16{u}_{h}")
            nc.vector.tensor_mul(t16[:, :w], lo[:, 0:w], rb[:, :w])
            cp(eng_oh[(u + h) % 2], oh8[:, h, nj0 : nj0 + w], t16[:, :w])
            eng_ol[(u + h) % 2].tensor_sub(
                ol8[:, h, nj0 : nj0 + w], t16[:, :w], oh8[:, h, nj0 : nj0 + w]
            )
            if h == QH - 1 and u >= 1:
                # Wo tiles fully covered by the PREVIOUS unit's columns (so
                # the fp8 o-splits they read are long since written)
                sP, kP, _ = units[u - 1]
                if kP == "pair" and T[sP] == 3:
                    nc_prev = nbase[sP] + 256
                else:
                    nc_prev = nbase[sP] + int(nhat[sP])
                emit_wo(nc_prev)

        LB, LC = 2, 4
        n_work = len(work)
        for idx in range(n_work + LC):
            if idx < n_work:
                stage_a(idx)
                flush_swap()
            if LB <= idx and idx - LB < n_work:
                stage_b(idx - LB)
            if LC <= idx and idx - LC < n_work:
                stage_c(idx - LC)
        emit_wo(NPACK)

    nc.compile()
    return nc


def _get_nc(T, nhat):
    key = (T, nhat)
    if key not in _CACHE:
        _CACHE[key] = _build_nc(T, nhat)
    return _CACHE[key]


def _split8(a):
    import ml_dtypes

    e4 = ml_dtypes.float8_e4m3
    hi = a.astype(e4)
    lo = (a - hi.astype(np.float32)).astype(e4)
    return hi, lo


def kernel(hidden_states, Wq, Wk, Wv, Wo, sid, position_ids):
    global LAST_EXEC_NS, LAST_RUN_WALL_S
    import time

    from concourse.bass_utils import run_bass_kernel_spmd

    hidden = np.asarray(hidden_states, dtype=np.float32)
    Wq = np.asarray(Wq, dtype=np.float32)
    Wk = np.asarray(Wk, dtype=np.float32)
    Wv = np.asarray(Wv, dtype=np.float32)
    Wo = np.asarray(Wo, dtype=np.float32)
    sid = np.asarray(sid)
    position_ids = np.asarray(position_ids)

    T, nhat, perms, counts = _structure(sid)
    nc = _get_nc(T, nhat)

    NBLK = sum(T)
    NPACK = int(sum(nhat))
    NT = (NPACK + 127) // 128
    nbase = np.cumsum([0] + list(nhat)).tolist()
    NMASK = 1 + NSEG

    f16 = np.float16

    # constants shared by all cores
    swpn = np.zeros((128, 128), f16)
    swpn[(np.arange(128) + 64) % 128, np.arange(128)] = 1.0
    idnn = np.eye(128, dtype=f16)
    ki = np.arange(128)[:, None]
    qi = np.arange(128)[None, :]
    tri = (ki <= qi).astype(f16)
    trin = np.zeros((128, NMASK, 128), f16)
    trin[:, 0, :] = tri
    for s in range(NSEG):
        rem = int(nhat[s]) - (T[s] - 1) * 128
        trin[:, 1 + s, :] = tri * (ki < rem)

    # weights per TP group (shared across batches)
    wgrp = []
    for g in range(TP):
        wq_dev = np.ascontiguousarray(
            (SW * Wq[g * 512 : (g + 1) * 512]).T
        ).reshape(NHC, 128, QH * D)
        wk_dev = np.ascontiguousarray(
            (SW * Wk[g * 128 : (g + 1) * 128]).T
        ).reshape(NHC, 128, D)
        wv_dev = np.ascontiguousarray(
            (SW * Wv[g * 128 : (g + 1) * 128]).T
        ).reshape(NHC, 128, D)
        # wo8[p, h, n] = SWO * Wo[n, g*512 + h*128 + p]
        wo_dev = np.ascontiguousarray(
            (SWO * Wo[:, g * 512 : (g + 1) * 512]).T.reshape(QH, 128, HID)
        ).transpose(1, 0, 2)
        ws = {}
        for name, a in (("wq8", wq_dev), ("wk8", wk_dev), ("wv8", wv_dev)):
            hi, lo = _split8(np.ascontiguousarray(a.transpose(1, 0, 2)))
            ws[name + "h"], ws[name + "l"] = hi, lo
        hi, lo = _split8(np.ascontiguousarray(wo_dev))
        ws["wo8h"], ws["wo8l"] = hi, lo
        wgrp.append(ws)

    in_maps = []
    real_rows = []
    for b in range(B):
        perm = perms[b]
        n_b = counts[b]
        # n-hat-packed x with zero fill between n_b and nhat
        xs = hidden[b].T[:, perm]  # [HID, L] sorted
        xpack = np.zeros((HID, NPACK), np.float32)
        pos = np.zeros(NPACK, np.float32)
        rows = []
        off = 0
        for s in range(NSEG):
            w = int(n_b[s])
            xpack[:, nbase[s] : nbase[s] + w] = xs[:, off : off + w] * SX
            pos[nbase[s] : nbase[s] + w] = position_ids[b][
                perm[off : off + w]
            ].astype(np.float32)
            rows.append(nbase[s] + np.arange(w))
            off += w
        real_rows.append(np.concatenate(rows))

        x8h, x8l = _split8(
            np.ascontiguousarray(xpack.reshape(NHC, 128, NPACK).transpose(1, 0, 2))
        )

        inv = 1.0 / (
            THETA ** (np.arange(0, D, 2, dtype=np.float32) / np.float32(D))
        )
        fr = pos[:, None] * inv[None, :]
        emb = np.concatenate([fr, fr], axis=1)  # [NPACK, D]
        cosT = np.ascontiguousarray(np.cos(emb).T.astype(f16))
        sinT = np.sin(emb).T.astype(np.float32).copy()
        sinT[: D // 2] *= -1.0  # fold rotate_half sign
        sinT = np.ascontiguousarray(sinT.astype(f16))

        for g in range(TP):
            m = dict(
                x8h=x8h,
                x8l=x8l,
                cosd=cosT,
                sind=sinT,
                trid=trin,
                idnd=idnn,
                swpd=swpn,
            )
            m.update(wgrp[g])
            in_maps.append(m)

    t0 = time.time()
    res = run_bass_kernel_spmd(nc, in_maps, core_ids=list(range(NCORES)))
    LAST_RUN_WALL_S = time.time() - t0
    LAST_EXEC_NS = res.exec_time_ns

    full = np.empty((B, L, HID), np.float32)
    for b in range(B):
        acc = np.asarray(res.results[4 * b]["out"]).astype(np.float32)
        for g in range(1, TP):
            acc += np.asarray(res.results[4 * b + g]["out"]).astype(np.float32)
        unp = np.empty((L, HID), np.float32)
        unp[perms[b]] = acc[real_rows[b]]
        full[b] = unp * OUT_DESCALE
    return full
